# revision 1
# baseline (speedup 1.0000x reference)
"""GCNConv (SpMM + dense projection) Trainium2 Bass kernel, 8-core SPMD.

Math: out = A @ x @ W, A symmetric COO (row, col, values), N=100000 nodes,
F=128 features, 1.6M edges.

Distribution (CAGNET-style 1D row partition): core m owns destination rows
[m*12500, (m+1)*12500). x is replicated in every core's HBM; each core
gathers the source rows its edges need via dma_gather (fp16, 256B rows).

Per-core device pipeline (single pass, pipelined by Tile):
  1. dma_gather x[col] rows for a group of 512-dest windows, one call per
     (window-group, col-chunk) — gather indices are int16 so x is split in
     4 chunks of 25000 rows.
  2. Per 128-edge tile, build the scatter matrix
     S[k, j] = v_k * (winslot(dest_k) == j) (one fused DVE
     tensor_scalar(is_equal, mult) against an iota constant).
  3. PE matmul E_tile^T @ S accumulates z^T[feat, dest] into a PSUM region
     per 512-dest window (per-element has_written accumulation).
  4. Evict PSUM -> SBUF fp16 (scalar engine), multiply by W (PE, W
     stationary) -> out^T, evict, DMA out.

Host work: window load balancing (so all cores share one static tile
schedule), bucketing edges by (core, window, chunk), packing the gather
index / slot / value streams, and the final unpermute + fp32 cast.
The measured bottleneck is SWDGE descriptor generation (~4.6 ns/row on
the GpSimd Q7 pair); everything else is hidden under it.
"""
import sys

if "/opt/trn_rl_repo" not in sys.path:
    sys.path.insert(0, "/opt/trn_rl_repo")

import numpy as np
from contextlib import ExitStack

import concourse.bacc as bacc
import concourse.tile as tile
import concourse.mybir as mybir
from concourse import bass_utils

F16 = mybir.dt.float16
F32 = mybir.dt.float32
I16 = mybir.dt.int16

# ---------------------------------------------------------------- config ---
DEFAULT_CFG = dict(
    n_nodes=100000,
    feat=128,
    n_cores=8,
    npc=12500,       # destination rows per core
    n_chunk=4,       # x row chunks (gather idx must fit int16)
    ch_rows=25000,   # rows per chunk
    wdest=512,       # dests per window (= one PSUM bank of fp32)
    nw=26,           # windows per core (26*512 = 13312 >= 12500)
    wpair=2,         # windows whose gathers are batched into one call
)


# ------------------------------------------------------- host preprocessing
def _assign_windows(deg4, nw, wdest):
    """Balanced assignment of destinations to nw windows (<= wdest each).

    Batched LPT: heaviest remaining dests go to the windows with the
    smallest worst-chunk load. Returns (win, slot) per destination.
    """
    npc = deg4.shape[0]
    tot = deg4.sum(axis=1)
    order = np.argsort(-tot, kind="stable")
    win = np.empty(npc, np.int32)
    slot = np.empty(npc, np.int32)
    loads = np.zeros((nw, 4), np.int64)
    nslot = np.zeros(nw, np.int32)
    pos = 0
    while pos < npc:
        k = min(nw, npc - pos)
        batch = order[pos : pos + k]
        wsel = np.argsort(loads.max(axis=1), kind="stable")[:k].astype(np.int32)
        win[batch] = wsel
        slot[batch] = nslot[wsel]
        nslot[wsel] += 1
        loads[wsel] += deg4[batch]
        pos += k
    assert nslot.max() <= wdest, f"window overflow: {nslot.max()}"
    return win, slot


def _preprocess(row, col, values, cfg):
    """Bucket edges per (core, window, chunk); compute the shared static tile
    schedule T[w][c]; pack per-core gather/slot/value streams."""
    nc_ = cfg["n_cores"]
    npc = cfg["npc"]
    chr_ = cfg["ch_rows"]
    nw = cfg["nw"]
    wdest = cfg["wdest"]
    wpair = cfg["wpair"]

    core = row // npc
    per_core = []
    for m in range(nc_):
        sel = np.flatnonzero(core == m)
        dl = (row[sel] - m * npc).astype(np.int64)
        cc = (col[sel] // chr_).astype(np.int64)
        lc = (col[sel] - cc * chr_).astype(np.int64)
        vv = values[sel].astype(np.float32)
        deg4 = np.bincount(dl * 4 + cc, minlength=npc * 4).reshape(npc, 4)
        win, slot = _assign_windows(deg4, nw, wdest)
        counts = np.bincount(
            win[dl].astype(np.int64) * 4 + cc, minlength=nw * 4
        ).reshape(nw, 4)
        per_core.append(dict(dl=dl, cc=cc, lc=lc, vv=vv, win=win, slot=slot,
                             counts=counts))

    # shared static schedule
    cmax = np.stack([pc["counts"] for pc in per_core]).max(axis=0)
    T = np.maximum((cmax + 127) // 128, 1).astype(np.int64)  # [nw, 4] tiles

    # stream layout: for window-pair b, for chunk c, for w in pair: T[w][c]
    n_batch = nw // wpair
    offs = np.zeros((nw, 4), np.int64)
    call_tiles = np.zeros((n_batch, 4), np.int64)
    cum = 0
    for b in range(n_batch):
        for c in range(4):
            for w in range(b * wpair, (b + 1) * wpair):
                offs[w, c] = cum
                cum += T[w, c]
            call_tiles[b, c] = cum - offs[b * wpair, c]
    tiles = int(cum)

    streams = []
    for m in range(nc_):
        pc = per_core[m]
        dl, cc, lc, vv = pc["dl"], pc["cc"], pc["lc"], pc["vv"]
        win, slot = pc["win"], pc["slot"]
        key = win[dl].astype(np.int64) * 4 + cc
        order = np.argsort(key, kind="stable")
        skey = key[order]
        starts = np.searchsorted(skey, np.arange(nw * 4))
        rank = np.arange(len(skey)) - starts[skey]
        gslot = offs.reshape(-1)[skey] * 128 + rank
        assert (rank < T.reshape(-1)[skey] * 128).all()

        idx_s = np.zeros(tiles * 128, np.int16)
        l_s = np.zeros(tiles * 128, np.float32)
        v_s = np.zeros(tiles * 128, np.float32)
        idx_s[gslot] = lc[order].astype(np.int16)
        l_s[gslot] = slot[dl][order].astype(np.float32)
        v_s[gslot] = vv[order]

        gidx = np.tile(np.ascontiguousarray(idx_s.reshape(-1, 16).T), (8, 1))
        lcol = np.ascontiguousarray(l_s.reshape(tiles, 128).T)
        vcol = np.ascontiguousarray(v_s.reshape(tiles, 128).T)

        destmap = -np.ones(nw * wdest, np.int64)
        destmap[win.astype(np.int64) * wdest + slot] = np.arange(npc)
        streams.append(dict(gidx=gidx, lcol=lcol, vcol=vcol, destmap=destmap))

    return T, offs, call_tiles, tiles, streams


# ------------------------------------------------------------ device build
def _build_program(T, call_tiles, tiles, cfg):
    nc_ = cfg["n_cores"]
    nw = cfg["nw"]
    wdest = cfg["wdest"]
    wpair = cfg["wpair"]
    nf = cfg["feat"]
    chr_ = cfg["ch_rows"]
    n_batch = nw // wpair

    nc = bacc.Bacc(
        "TRN2",
        debug=False,
        target_bir_lowering=False,
        num_devices=nc_,
        num_swdge_queues=4,
    )
    x16 = nc.dram_tensor("x16", [cfg["n_nodes"], nf], F16, kind="ExternalInput")
    w16 = nc.dram_tensor("w16", [nf, nf], F16, kind="ExternalInput")
    iota = nc.dram_tensor("iota", [128, wdest], F16, kind="ExternalInput")
    gidx = nc.dram_tensor("gidx", [128, tiles * 8], I16, kind="ExternalInput")
    lcol = nc.dram_tensor("lcol", [128, tiles], F32, kind="ExternalInput")
    vcol = nc.dram_tensor("vcol", [128, tiles], F32, kind="ExternalInput")
    outT = nc.dram_tensor("outT", [128, nw * wdest], F16, kind="ExternalOutput")

    with tile.TileContext(nc) as tc, ExitStack() as ctx:
        const = ctx.enter_context(tc.tile_pool(name="const", bufs=1))
        gpools = [
            ctx.enter_context(tc.tile_pool(name=f"g{c}", bufs=2))
            for c in range(4)
        ]
        spool = ctx.enter_context(tc.tile_pool(name="s", bufs=6))
        pspool = ctx.enter_context(tc.tile_pool(name="ps", bufs=4, space="PSUM"))
        pzpool = ctx.enter_context(tc.tile_pool(name="pz", bufs=2, space="PSUM"))
        zbpool = ctx.enter_context(tc.tile_pool(name="zb", bufs=3))
        zopool = ctx.enter_context(tc.tile_pool(name="zo", bufs=3))

        iota_t = const.tile([128, wdest], F16)
        nc.sync.dma_start(iota_t[:], iota[:, :])
        idx_t = const.tile([128, tiles * 8], I16)
        nc.sync.dma_start(idx_t[:], gidx[:, :])
        w_t = const.tile([128, nf], F16)
        nc.sync.dma_start(w_t[:], w16[:, :])
        l_t = const.tile([128, tiles], F32)
        nc.sync.dma_start(l_t[:], lcol[:, :])
        v_t = const.tile([128, tiles], F32)
        nc.sync.dma_start(v_t[:], vcol[:, :])

        for b in range(n_batch):
            ws = list(range(b * wpair, (b + 1) * wpair))
            # gather: one call per chunk covering the window pair
            gts = []
            for c in range(4):
                sz = int(call_tiles[b, c])
                gt = gpools[c].tile([128, sz, nf], F16, tag=f"g{c}")
                t0 = int(np.sum(call_tiles[:b]) + np.sum(call_tiles[b, :c]))
                nc.gpsimd.dma_gather(
                    gt[:, :, :],
                    x16[c * chr_ : (c + 1) * chr_, :],
                    idx_t[:, t0 * 8 : (t0 + sz) * 8],
                    sz * 128,
                    sz * 128,
                    nf,
                    queue_num=c,
                    single_packet=False,
                )
                gts.append((gt, t0))

            for wp in range(wpair):
                w = ws[wp]
                ps = pspool.tile([128, wdest], F32, tag="ps")
                first = True
                last_ct = None
                for c in range(3, -1, -1):
                    if T[w, c] > 0:
                        last_ct = (c, int(T[w, c]) - 1)
                        break
                for c in range(4):
                    gt, t0 = gts[c]
                    base = int(np.sum([T[ws[i], c] for i in range(wp)]))
                    for t in range(int(T[w, c])):
                        g = t0 + base + t
                        s_t = spool.tile([128, wdest], F16, tag="s")
                        nc.vector.tensor_scalar(
                            s_t[:],
                            iota_t[:],
                            l_t[:, g : g + 1],
                            v_t[:, g : g + 1],
                            mybir.AluOpType.is_equal,
                            mybir.AluOpType.mult,
                        )
                        nc.tensor.matmul(
                            ps[:],
                            gt[:, base + t, :],
                            s_t[:],
                            start=first,
                            stop=(c, t) == last_ct,
                        )
                        first = False

                zb = zbpool.tile([128, wdest], F16, tag="zb")
                nc.scalar.copy(zb[:], ps[:])
                pz = pzpool.tile([128, wdest], F32, tag="pz")
                nc.tensor.matmul(pz[:], w_t[:], zb[:], start=True, stop=True)
                zo = zopool.tile([128, wdest], F16, tag="zo")
                nc.scalar.copy(zo[:], pz[:])
                nc.sync.dma_start(
                    outT[:, w * wdest : (w + 1) * wdest], zo[:]
                )

    nc.compile()
    return nc


# ------------------------------------------------------------------- entry
def _run(row, col, values, x, weight, cfg, trace=False):
    row = np.asarray(row, dtype=np.int64)
    col = np.asarray(col, dtype=np.int64)
    values = np.asarray(values, dtype=np.float32)
    x = np.asarray(x, dtype=np.float32)
    weight = np.asarray(weight, dtype=np.float32)

    nc_ = cfg["n_cores"]
    npc = cfg["npc"]

    T, offs, call_tiles, tiles, streams = _preprocess(row, col, values, cfg)
    nc = _build_program(T, call_tiles, tiles, cfg)

    x16 = x.astype(np.float16)
    w16 = weight.astype(np.float16)
    iota_np = np.tile(
        np.arange(cfg["wdest"], dtype=np.float16)[None, :], (128, 1)
    )

    in_maps = []
    for m in range(nc_):
        st = streams[m]
        in_maps.append(
            dict(x16=x16, w16=w16, iota=iota_np, gidx=st["gidx"],
                 lcol=st["lcol"], vcol=st["vcol"])
        )

    res = bass_utils.run_bass_kernel_spmd(
        nc, in_maps, core_ids=list(range(nc_)), trace=trace
    )

    out = np.zeros((cfg["n_nodes"], cfg["feat"]), np.float32)
    for m in range(nc_):
        oT = res.results[m]["outT"].astype(np.float32)  # [128, nw*wdest]
        dm = streams[m]["destmap"]
        valid = dm >= 0
        out[m * npc + dm[valid]] = oT[:, valid].T
    return out, res


def kernel(row, col, values, x, weight):
    out, _ = _run(row, col, values, x, weight, DEFAULT_CFG)
    return out



# revision 2
# speedup vs baseline: 1.2811x; 1.2811x over previous
"""GCNConv (SpMM + dense projection) Trainium2 Bass kernel, 8-core SPMD. V2.

out = A @ x @ W, A symmetric COO (1.6M edges), N=100K nodes, F=128.

Core m owns dests [m*12500, (m+1)*12500), split into 98 windows of 128
dests. Rows gathered per unique (col, window) incidence (dma_gather,
SWDGE, 4 queues round-robin). Scatter matrices S (dense [128 rows x 128
dests] fp8, multiple edges per row folded) are host-built and streamed
from HBM, so the Vector engine does nothing. Per tile: PE matmul
z^T[f,d] += E^T-tile. After a window's last tile: evict z^T, project by
W, write out^T.

Shared static schedule: per-(window, chunk) run lengths are cross-core
maxima; per-core streams are padded to match.
"""
import sys

if "/opt/trn_rl_repo" not in sys.path:
    sys.path.insert(0, "/opt/trn_rl_repo")

import numpy as np
from contextlib import ExitStack

import concourse.bacc as bacc
import concourse.tile as tile
import concourse.mybir as mybir
from concourse import bass_utils
import ml_dtypes

F16 = mybir.dt.float16
F32 = mybir.dt.float32
F8 = mybir.dt.float8e4
I16 = mybir.dt.int16
NP_F8 = ml_dtypes.float8_e4m3

DEFAULT_CFG = CFG = dict(
    n_nodes=100000,
    feat=128,
    n_cores=8,
    npc=12500,
    wdest=128,
    nw=98,        # 98*128 = 12544 >= 12500
    gw=7,         # windows per group (one PSUM bank each)
    ngroup=14,
    n_chunk=4,
    ch_rows=25000,
)


def _preprocess(row, col, values, cfg):
    nc_ = cfg["n_cores"]
    npc = cfg["npc"]
    nw = cfg["nw"]
    wdest = cfg["wdest"]
    gw = cfg["gw"]
    ngroup = cfg["ngroup"]
    nch = cfg["n_chunk"]
    chr_ = cfg["ch_rows"]

    core = row // npc
    per_core = []
    for m in range(nc_):
        sel = np.flatnonzero(core == m)
        dl = (row[sel] - m * npc).astype(np.int64)
        w = dl // wdest
        d = dl % wdest
        c = (col[sel] // chr_).astype(np.int64)
        lc = (col[sel] - c * chr_).astype(np.int64)
        vv = values[sel].astype(np.float32)
        # unique (w, c, lc) incidences, and edge -> incidence rank
        key = (w * nch + c) * chr_ + lc
        order = np.argsort(key, kind="stable")
        skey = key[order]
        uniq, inv_sorted, counts = np.unique(
            skey, return_inverse=True, return_counts=True)
        inv = np.empty(len(skey), np.int64)
        inv[order] = inv_sorted
        # incidence -> (w, c, lc)
        u_w = uniq // (nch * chr_)
        u_c = (uniq // chr_) % nch
        u_lc = uniq % chr_
        # L[w, c] = number of incidences
        L = np.bincount(u_w * nch + u_c, minlength=nw * nch).reshape(nw, nch)
        per_core.append(dict(w=w, d=d, vv=vv, inv=inv,
                             u_w=u_w, u_c=u_c, u_lc=u_lc, L=L))

    Lbar = np.stack([pc["L"] for pc in per_core]).max(axis=0)  # [nw, nch]

    # call (g, c): rows = sum over w in group of Lbar[w, c], padded to 128
    call_rows = np.zeros((ngroup, nch), np.int64)
    run_off = np.zeros((nw, nch), np.int64)  # offset of (w,c) run inside call
    for g in range(ngroup):
        for c in range(nch):
            off = 0
            for w in range(g * gw, (g + 1) * gw):
                run_off[w, c] = off
                off += int(Lbar[w, c])
            call_rows[g, c] = -(-off // 128) * 128
    sz = (call_rows // 128).astype(np.int64)   # tiles per call

    # schedule: per call, per tile, windows intersecting it (shared)
    # pair list: (g, c, t, w) ; pair id ordered by (g, c, t, w)
    pairs = []
    pair_id = {}
    for g in range(ngroup):
        for c in range(nch):
            bounds = []  # (w, start_row, end_row)
            for w in range(g * gw, (g + 1) * gw):
                if Lbar[w, c] > 0:
                    s = run_off[w, c]
                    bounds.append((w, s, s + int(Lbar[w, c])))
            for t in range(int(sz[g, c])):
                lo, hi = t * 128, (t + 1) * 128
                for (w, s, e) in bounds:
                    if s < hi and e > lo:
                        pair_id[(g, c, t, w)] = len(pairs)
                        pairs.append((g, c, t, w))
    ns = len(pairs)

    # start/stop flags per pair (first/last pair of each window)
    first_pair = {}
    last_pair = {}
    for i, (g, c, t, w) in enumerate(pairs):
        if (g, w) not in first_pair:
            first_pair[(g, w)] = i
        last_pair[(g, w)] = i

    # per-core streams
    streams = []
    total_rows = int(call_rows.sum())
    for m in range(nc_):
        pc = per_core[m]
        u_w, u_c, u_lc, L = pc["u_w"], pc["u_c"], pc["u_lc"], pc["L"]
        # incidence -> global stream row
        # rows of (w,c) run start at call_base[g,c] + run_off[w,c]
        call_base = np.zeros((ngroup, nch), np.int64)
        acc = 0
        for g in range(ngroup):
            for c in range(nch):
                call_base[g, c] = acc
                acc += int(call_rows[g, c])
        # incidence rank within its (w,c) run (u_* sorted by key = (w,c,lc))
        wc = u_w * nch + u_c
        run_starts = np.searchsorted(wc, np.arange(nw * nch))
        rank = np.arange(len(wc)) - run_starts[wc]
        g_of_w = u_w // gw
        srow = (call_base[g_of_w, u_c] + run_off[u_w, u_c] + rank)

        idx_s = np.zeros(total_rows, np.int16)
        idx_s[srow] = u_lc.astype(np.int16)

        # gather idx packed per call: [128, total_rows//16] int16
        gidx = np.tile(
            np.ascontiguousarray(idx_s.reshape(-1, 16).T), (8, 1))

        # S stream: [ns, 128, 128] fp32 -> fp8
        S = np.zeros((ns, 128, 128), np.float32)
        w_e, d_e, vv, inv = pc["w"], pc["d"], pc["vv"], pc["inv"]
        srow_e = srow[inv]            # edge -> stream row
        g_e = w_e // gw
        c_e = u_c[inv]
        t_e = (srow_e - call_base[g_e, c_e]) // 128
        r_e = srow_e % 128
        p_e = np.array([pair_id[(g, c, t, w)] for (g, c, t, w) in
                        zip(g_e.tolist(), c_e.tolist(), t_e.tolist(),
                            w_e.tolist())], np.int64)
        np.add.at(S, (p_e, r_e, d_e), vv)
        # S tensor layout [128 rows, ns*128 dests]
        S8 = np.ascontiguousarray(
            S.transpose(1, 0, 2).reshape(128, ns * 128)).astype(np.float16)

        streams.append(dict(gidx=gidx, S8=S8))

    sched = dict(Lbar=Lbar, call_rows=call_rows, sz=sz, pairs=pairs,
                 first_pair=first_pair, last_pair=last_pair, ns=ns,
                 total_rows=total_rows)
    return sched, streams


def _build_program(sched, cfg):
    nc_ = cfg["n_cores"]
    nw = cfg["nw"]
    wdest = cfg["wdest"]
    gw = cfg["gw"]
    ngroup = cfg["ngroup"]
    nch = cfg["n_chunk"]
    chr_ = cfg["ch_rows"]
    nf = cfg["feat"]

    sz = sched["sz"]
    pairs = sched["pairs"]
    first_pair = sched["first_pair"]
    last_pair = sched["last_pair"]
    ns = sched["ns"]
    total_rows = sched["total_rows"]

    nc = bacc.Bacc("TRN2", debug=False, target_bir_lowering=False,
                   num_devices=nc_, num_swdge_queues=4)
    x16 = nc.dram_tensor("x16", [cfg["n_nodes"], nf], F16,
                         kind="ExternalInput")
    w16 = nc.dram_tensor("w16", [nf, nf], F16, kind="ExternalInput")
    gidx = nc.dram_tensor("gidx", [128, total_rows // 16], I16,
                          kind="ExternalInput")
    sten = nc.dram_tensor("sten", [128, ns * wdest], F16,
                          kind="ExternalInput")
    outT = nc.dram_tensor("outT", [128, nw * wdest], F16,
                          kind="ExternalOutput")

    # group the pair list by (g, c): matmuls in program order
    pairs_by_call = {}
    for i, (g, c, t, w) in enumerate(pairs):
        pairs_by_call.setdefault((g, c), []).append((i, t, w))

    with tile.TileContext(nc) as tc, ExitStack() as ctx:
        const = ctx.enter_context(tc.tile_pool(name="const", bufs=1))
        gpools = [
            ctx.enter_context(tc.tile_pool(name=f"g{c}", bufs=2))
            for c in range(nch)
        ]
        spool = ctx.enter_context(tc.tile_pool(name="s", bufs=3))
        pspool = ctx.enter_context(tc.tile_pool(name="ps", bufs=1,
                                                space="PSUM"))
        pzpool = ctx.enter_context(tc.tile_pool(name="pz", bufs=1,
                                                space="PSUM"))
        zbpool = ctx.enter_context(tc.tile_pool(name="zb", bufs=3))
        zopool = ctx.enter_context(tc.tile_pool(name="zo", bufs=3))

        idx_t = const.tile([128, total_rows // 16], I16)
        nc.sync.dma_start(idx_t[:], gidx[:, :])
        w_t = const.tile([128, nf], F16)
        nc.sync.dma_start(w_t[:], w16[:, :])

        row_base = np.zeros((ngroup, nch), np.int64)
        acc = 0
        for g in range(ngroup):
            for c in range(nch):
                row_base[g, c] = acc
                acc += int(sz[g, c]) * 128

        # S stream base per call (pairs are call-ordered)
        s_base = {}
        acc = 0
        for g in range(ngroup):
            for c in range(nch):
                s_base[(g, c)] = acc
                acc += len(pairs_by_call.get((g, c), []))

        call_idx = 0
        for g in range(ngroup):
            # PSUM tiles for this group's windows (one bank each)
            ps = {}
            for w in range(g * gw, (g + 1) * gw):
                pt = pspool.tile([128, 512], F32, tag=f"ps{w % gw}")
                ps[w] = pt
            gts = {}
            for c in range(nch):
                szc = int(sz[g, c])
                if szc == 0:
                    continue
                gt = gpools[c].tile([128, szc, nf], F16, tag=f"g{c}")
                t0 = int(row_base[g, c])
                nc.gpsimd.dma_gather(
                    gt[:, :, :],
                    x16[c * chr_: (c + 1) * chr_, :],
                    idx_t[:, t0 // 16: (t0 + szc * 128) // 16],
                    szc * 128, szc * 128, nf,
                    queue_num=call_idx % 4,
                    single_packet=False,
                )
                call_idx += 1
                gts[c] = gt

            for c in range(nch):
                plist = pairs_by_call.get((g, c), [])
                if not plist:
                    continue
                gt = gts[c]
                sb = s_base[(g, c)]
                n_call = len(plist)
                st = spool.tile([128, n_call * wdest], F16, tag="s")
                nc.sync.dma_start(
                    st[:], sten[:, sb * wdest: (sb + n_call) * wdest])
                for j, (i, t, w) in enumerate(plist):
                    nc.tensor.matmul(
                        ps[w][:, 0:wdest],
                        gt[:, t, :],
                        st[:, j * wdest: (j + 1) * wdest],
                        start=(first_pair[(g, w)] == i),
                        stop=(last_pair[(g, w)] == i),
                    )

            # project + evict each window of the group
            for w in range(g * gw, (g + 1) * gw):
                zb = zbpool.tile([128, wdest], F16, tag="zb")
                nc.scalar.copy(zb[:], ps[w][:, 0:wdest])
                pz = pzpool.tile([128, 512], F32, tag="pz")
                nc.tensor.matmul(pz[:, 0:wdest], w_t[:], zb[:], start=True,
                                 stop=True)
                zo = zopool.tile([128, wdest], F16, tag="zo")
                nc.scalar.copy(zo[:], pz[:, 0:wdest])
                nc.sync.dma_start(
                    outT[:, w * wdest: (w + 1) * wdest], zo[:])

    nc.compile()
    return nc


def _run(row, col, values, x, weight, cfg, trace=False):
    row = np.asarray(row, dtype=np.int64)
    col = np.asarray(col, dtype=np.int64)
    values = np.asarray(values, dtype=np.float32)
    x = np.asarray(x, dtype=np.float32)
    weight = np.asarray(weight, dtype=np.float32)

    nc_ = cfg["n_cores"]
    npc = cfg["npc"]

    sched, streams = _preprocess(row, col, values, cfg)
    nc = _build_program(sched, cfg)

    x16 = x.astype(np.float16)
    w16 = weight.astype(np.float16)

    in_maps = []
    for m in range(nc_):
        st = streams[m]
        in_maps.append(dict(x16=x16, w16=w16, gidx=st["gidx"],
                            sten=st["S8"]))

    res = bass_utils.run_bass_kernel_spmd(
        nc, in_maps, core_ids=list(range(nc_)), trace=trace
    )

    out = np.zeros((cfg["n_nodes"], cfg["feat"]), np.float32)
    for m in range(nc_):
        oT = res.results[m]["outT"].astype(np.float32)
        out[m * npc: (m + 1) * npc] = oT[:, :npc].T
    return out, res


def kernel(row, col, values, x, weight):
    out, _ = _run(row, col, values, x, weight, CFG)
    return out
